# revision 29
# baseline (speedup 1.0000x reference)
"""
Trainium2 Bass kernel for nn_Attention (dense transformer attention block).

Model (reference):
  qh = ((q+qpos) @ wq.T + bq)   -> heads
  kh = ((k+kpos) @ wk.T + bk)
  vh = (v @ wv.T + bv)
  attn = softmax(mask(qh kh^T * scale)) ; x = attn @ vh ; out = x @ proj.T + pb

Sharding (8 cores): hybrid batch x head-group.  core c -> batch b=c//4,
head-group g=c%4 (4 heads = 256 dims of the 1024 hidden dim).  Each core:
  - QKV projections for its 256-dim slice over its batch's 2048 tokens
  - attention for its 4 heads (fully local QK^T/softmax/AV, causal blocks only)
  - partial output projection  y_c = attn_x[:, 256g:256g+256] @ proj_w[:,sl].T
Host: y[b] = sum over the 4 cores of batch b  (Megatron-style partial sum) + pb.

All matmul inputs are bf16 (PSUM accumulation fp32); activations stream to the
device as bf16, halving HBM traffic and DVE element cost.  Projections run
K-contiguous (contraction-inner per output tile) so a single PSUM bank is live
at a time and the PE stays warm.  Softmax uses no max-subtraction (scores are
O(5); exp is safe in fp32).  P = exp(scale*S) is produced by one merged ACT op
per (k-tile, head-pair): head0 scores at [cs:512], head1 at [512:1024-cs], so
the exp region [cs:1024-cs] is contiguous and exactly the causally-live part.
Denominators come free from the AV matmul via a ones-column per head in the
VH tiles ([128, 4*65]); AV psum row 64 is the per-(head,q) colsum.
"""

import sys
import numpy as np

for _p in ("/opt/trn_rl_repo",):
    if _p not in sys.path:
        sys.path.insert(0, _p)

import ml_dtypes

import concourse.bass as bass
import concourse.bacc as bacc
import concourse.mybir as mybir
import concourse.tile as tile
from concourse.bass import ts
from concourse.bass_utils import run_bass_kernel_spmd

F32 = mybir.dt.float32
F32R = mybir.dt.float32r
BF16 = mybir.dt.bfloat16
EXP = mybir.ActivationFunctionType.Exp
BF16NP = ml_dtypes.bfloat16

HID = 1024          # hidden dim
DS = 256            # per-core dim slice (4 heads x 64)
NT = 2048           # tokens per batch
HD = 64             # head dim
NHEADS_CORE = 4
SCALE = HD ** -0.5
NKT = HID // 128    # hidden contraction tiles
NTOK = NT // 128    # token tiles of 128
NQC = NT // 512     # 512-wide token chunks
VW = NHEADS_CORE * 65   # VH-augmented tile width (64 data + 1 ones per head)

_NC_CACHE = {}


def _build_nc(phases=("qk", "v", "att", "proj"), reps=1):
    from contextlib import ExitStack

    nc = bacc.Bacc(num_swdge_queues=4)
    xqT = nc.declare_dram_parameter("xqT", [HID, NT], BF16, isOutput=False)
    xkT = nc.declare_dram_parameter("xkT", [HID, NT], BF16, isOutput=False)
    vT = nc.declare_dram_parameter("vT", [HID, NT], BF16, isOutput=False)
    wqT = nc.declare_dram_parameter("wqT", [128, NKT, DS], BF16, isOutput=False)
    wkT = nc.declare_dram_parameter("wkT", [128, NKT, DS], BF16, isOutput=False)
    wvT = nc.declare_dram_parameter("wvT", [128, NKT, DS], BF16, isOutput=False)
    wqb = nc.declare_dram_parameter("wqb", [128, 2], F32, isOutput=False)
    wkb = nc.declare_dram_parameter("wkb", [128, 2], F32, isOutput=False)
    wvb = nc.declare_dram_parameter("wvb", [1, DS], BF16, isOutput=False)
    projT = nc.declare_dram_parameter("projT", [DS, HID], BF16, isOutput=False)
    maskp = nc.declare_dram_parameter("maskp", [128, 128], BF16, isOutput=False)
    y = nc.declare_dram_parameter("y", [NT, HID], BF16, isOutput=True)

    with tile.TileContext(nc) as tc, ExitStack() as ctx:
        ctx.enter_context(nc.allow_low_precision(
            reason="bf16 matmul inputs by design; fp32 PSUM accumulation"))
        pers = ctx.enter_context(tc.tile_pool(name="pers", bufs=1))

        QHT = [pers.tile([128, NT], BF16, tag=f"qht{i}", name=f"qht{i}")
               for i in range(2)]
        KHT = [pers.tile([128, NT], BF16, tag=f"kht{i}", name=f"kht{i}")
               for i in range(2)]
        AVN = [pers.tile([128, NT], BF16, tag=f"avn{i}", name=f"avn{i}")
               for i in range(2)]
        VH = [pers.tile([128, VW], BF16, tag=f"vh{m}", name=f"vh{m}")
              for m in range(NTOK)]

        wq_s = pers.tile([128, NKT, DS], BF16, tag="wq")
        wk_s = pers.tile([128, NKT, DS], BF16, tag="wk")
        wv_s = pers.tile([128, NKT, DS], BF16, tag="wv")
        pj_s = [pers.tile([128, HID], BF16, tag=f"pj{i}", name=f"pj{i}")
                for i in range(2)]
        mk_s = pers.tile([128, 128], BF16, tag="mask")
        qb_s = pers.tile([128, 2], F32, tag="wqb")
        kb_s = pers.tile([128, 2], F32, tag="wkb")
        vb_s = pers.tile([1, DS], BF16, tag="wvb")
        onesf = pers.tile([1, 128], F32, tag="onesf")
        ones_b = pers.tile([1, 128], BF16, tag="onesb")
        ones_r = pers.tile([1, 64], F32R, tag="onesr")

        nc.vector.memset(onesf[:], 1.0)
        nc.vector.tensor_copy(ones_b[:], onesf[:])
        nc.vector.tensor_copy(ones_r[:], onesf[0:1, 0:64])
        for m in range(NTOK):
            vh3 = VH[m].rearrange("p (h w) -> p h w", w=65)
            nc.vector.memset(vh3[:, :, 64:65], 1.0)

        # PSUM: proj/rnorm ring 2x2KB + av ring 2x2KB + score ring 2x4KB = 16KB
        ppp = ctx.enter_context(
            tc.tile_pool(name="ppp", bufs=2, space=bass.MemorySpace.PSUM))
        avp = ctx.enter_context(
            tc.tile_pool(name="avp", bufs=2, space=bass.MemorySpace.PSUM))
        spp = ctx.enter_context(
            tc.tile_pool(name="spp", bufs=2, space=bass.MemorySpace.PSUM))
        xsp = ctx.enter_context(tc.tile_pool(name="xsp", bufs=26))
        ptp = ctx.enter_context(tc.tile_pool(name="pt", bufs=8))
        asb = ctx.enter_context(tc.tile_pool(name="asb", bufs=6))
        ysb = ctx.enter_context(tc.tile_pool(name="ysb", bufs=6))

        nc.sync.dma_start(wv_s[:], wvT[:])
        nc.sync.dma_start(vb_s[:], wvb[:])
        nc.sync.dma_start(wq_s[:], wqT[:])
        nc.sync.dma_start(qb_s[:], wqb[:])
        nc.sync.dma_start(wk_s[:], wkT[:])
        nc.sync.dma_start(kb_s[:], wkb[:])
        nc.sync.dma_start(mk_s[:], maskp[:])
        for i in range(2):
            nc.sync.dma_start(pj_s[i][:], projT[ts(i, 128), :])

        # Software pipeline: attention(r) is emitted interleaved with the
        # projection work of rep r+1 as "filler" jobs popped once per i-loop
        # iteration.  The reversed qc order frees VH[4qc:4qc+4] and the
        # QHT/KHT q-chunk regions right after each qc block, so next-rep
        # projection tiles can be rebuilt while attention continues.  All
        # fillers depend only on already-emitted work, so they never stall
        # the in-order PE queue.
        fillers = []

        def pop_filler():
            if fillers:
                fillers.pop(0)()

        def emit_xloads(src):
            xs = []
            for kt in range(NKT):
                x = xsp.tile([128, NT], BF16, tag="xs", name="xs")
                nc.sync.dma_start(x[:, 0:NT // 2], src[ts(kt, 128), 0:NT // 2])
                nc.sync.dma_start(x[:, NT // 2:], src[ts(kt, 128), NT // 2:])
                xs.append(x)
            return xs

        def v_job(xs, m):
            def go():
                ps = ppp.tile([128, 512], F32, tag="pp", name="psV")
                for kt in range(NKT):
                    nc.tensor.matmul(ps[:, 0:DS], xs[kt][:, ts(m, 128)],
                                     wv_s[:, kt, :],
                                     start=(kt == 0), stop=False)
                nc.tensor.matmul(ps[:, 0:DS], ones_b[0:1, :], vb_s[0:1, :],
                                 start=False, stop=True)
                vh3 = VH[m].rearrange("p (h w) -> p h w", w=65)
                nc.vector.tensor_copy(
                    vh3[:, :, 0:64],
                    ps[:, 0:DS].rearrange("p (h w) -> p h w", w=64))
            return go

        def qk_job(xs, w_s, b_s, OUT, m, n2):
            def go():
                ps = ppp.tile([128, 512], F32, tag="pp", name="psA")
                for kt in range(NKT):
                    nc.tensor.matmul(ps[:], w_s[:, kt, ts(m, 128)],
                                     xs[kt][:, ts(n2, 512)],
                                     start=(kt == 0), stop=(kt == NKT - 1))
                nc.vector.tensor_scalar_add(OUT[m][:, ts(n2, 512)], ps[:],
                                            b_s[:, m:m + 1])
            return go

        def qk_job_halves(xs, w_s, b_s, OUT, m, n2):
            # two filler quanta sharing one accumulation tile; MUST stay
            # adjacent in the FIFO so no other ppp alloc lands between them
            st = {}

            def go1():
                st["ps"] = ppp.tile([128, 512], F32, tag="pp", name="psA")
                for kt in range(NKT // 2):
                    nc.tensor.matmul(st["ps"][:], w_s[:, kt, ts(m, 128)],
                                     xs[kt][:, ts(n2, 512)],
                                     start=(kt == 0), stop=False)

            def go2():
                for kt in range(NKT // 2, NKT):
                    nc.tensor.matmul(st["ps"][:], w_s[:, kt, ts(m, 128)],
                                     xs[kt][:, ts(n2, 512)],
                                     start=False, stop=(kt == NKT - 1))
                nc.vector.tensor_scalar_add(OUT[m][:, ts(n2, 512)],
                                            st["ps"][:], b_s[:, m:m + 1])
            return go1, go2

        def proj_job(m, n2):
            def go():
                ps = ppp.tile([128, 512], F32, tag="pp", name="psY")
                for kd in range(2):
                    nc.tensor.matmul(ps[:], AVN[kd][:, ts(m, 128)],
                                     pj_s[kd][:, ts(n2, 512)],
                                     start=(kd == 0), stop=(kd == 1))
                ys = ysb.tile([128, 512], BF16, tag="ys", name="ys")
                nc.vector.tensor_copy(ys[:], ps[:])
                nc.sync.dma_start(y[ts(m, 128), ts(n2, 512)], ys[:])
            return go

        # prologue: rep-0 projections emitted inline.  (Attention q-chunk 3
        # reads ALL KHT key regions from its first iteration, so K tiles
        # cannot be deferred into the attention stream for rep 0.)
        xsV = emit_xloads(vT)
        xsQ = emit_xloads(xqT)
        xsK = emit_xloads(xkT)
        for m in range(NTOK):
            v_job(xsV, m)()
        for m in range(2):
            for n2 in range(NQC):
                qk_job(xsQ, wq_s, qb_s, QHT, m, n2)()
        for m in range(2):
            for n2 in range(NQC):
                qk_job(xsK, wk_s, kb_s, KHT, m, n2)()

        for _rep in range(reps):
            nxt = _rep + 1 < reps
            if nxt:
                xsV = emit_xloads(vT)
                xsQ = emit_xloads(xqT)
                xsK = emit_xloads(xkT)
            if True:
                for qc in reversed(range(NQC)):
                    nkt = 4 * qc + 4        # causal: k-tiles 0..4qc+3
                    for ht in range(2):     # head pair (2*ht, 2*ht+1)
                        av = [avp.tile([65, 512], F32, tag="av", name="av")
                              for _ in range(2)]
                        for i in range(nkt):
                            d = i - 4 * qc
                            cs = 128 * d if d > 0 else 0
                            w = 512 - cs
                            sp2 = spp.tile([128, 1024], F32, tag="sp", name="sp")
                            # head0 scores at [cs:512], head1 at [512:1024-cs]
                            nc.tensor.matmul(
                                sp2[:, cs:512],
                                KHT[ht][0:HD, ts(i, 128)],
                                QHT[ht][0:HD, qc * 512 + cs:(qc + 1) * 512],
                                start=True, stop=True)
                            nc.tensor.matmul(
                                sp2[:, 512:512 + w],
                                KHT[ht][HD:128, ts(i, 128)],
                                QHT[ht][HD:128, qc * 512 + cs:(qc + 1) * 512],
                                start=True, stop=True)
                            pt2 = ptp.tile([128, 1024], BF16, tag="pt",
                                           name="pt")
                            nc.scalar.activation(pt2[:, cs:512 + w],
                                                 sp2[:, cs:512 + w], EXP,
                                                 scale=SCALE)
                            if d >= 0:
                                nc.vector.tensor_mul(
                                    pt2[:, cs:cs + 128],
                                    pt2[:, cs:cs + 128], mk_s[:])
                                nc.vector.tensor_mul(
                                    pt2[:, 512:512 + 128],
                                    pt2[:, 512:512 + 128], mk_s[:])
                            for sub in range(2):
                                h = 2 * ht + sub
                                rs = cs if sub == 0 else 512
                                nc.tensor.matmul(
                                    av[sub][:, cs:512],
                                    VH[i][:, 65 * h:65 * h + 65],
                                    pt2[:, rs:rs + w],
                                    start=(i == 0), stop=(i == nkt - 1))
                            # spread fillers over the remaining pop sites of
                            # this qc block so the queue never starves the
                            # ACT-paced stretches at the block tail
                            slots_left = (nkt - 1 - i) + 2 + \
                                (nkt + 4 if ht == 0 else 0)
                            if fillers and (i % 2 == 1
                                            or len(fillers) >= slots_left):
                                pop_filler()
                        for sub in range(2):
                            hp = sub * HD
                            rec = asb.tile([1, 512], F32R, tag="rec",
                                           name="rec")
                            nc.vector.reciprocal(rec[:], av[sub][64:65, :])
                            pop_filler()
                            rp = ppp.tile([64, 512], F32, tag="pp", name="rp")
                            nc.tensor.matmul(rp[:], ones_r[0:1, :], rec[:],
                                             start=True, stop=True)
                            rps = asb.tile([64, 512], F32, tag="rps",
                                           name="rps")
                            nc.vector.tensor_copy(rps[:], rp[:])
                            nc.vector.tensor_mul(
                                AVN[ht][hp:hp + HD, ts(qc, 512)],
                                av[sub][0:64, :], rps[:])
                    # queue next-rep projection work freed by this qc block,
                    # interleaving big (V/QK, 1-2us) and small (proj, 0.4us)
                    # jobs so the per-iteration injected PE work is smooth
                    small = [proj_job(4 * qc + mi, n2)
                             for mi in range(4) for n2 in range(2)]
                    if nxt:
                        big = [v_job(xsV, m)
                               for m in range(4 * qc, 4 * qc + 4)]
                        for m in range(2):
                            big += list(qk_job_halves(xsQ, wq_s, qb_s, QHT,
                                                      m, qc))
                        for m in range(2):
                            big += list(qk_job_halves(xsK, wk_s, kb_s, KHT,
                                                      m, qc))
                        mixed = []
                        while big or small:
                            if big:
                                mixed.append(big.pop(0))
                            if small:
                                mixed.append(small.pop(0))
                        fillers.extend(mixed)
                    else:
                        fillers.extend(small)
        while fillers:
            fillers.pop(0)()

    nc.compile()
    return nc


def _get_nc():
    if "nc" not in _NC_CACHE:
        _NC_CACHE["nc"] = _build_nc()
    return _NC_CACHE["nc"]


def make_in_maps(q, k, v, qpos, kpos, mask, wq_w, wq_b, wk_w, wk_b, wv_w, wv_b,
                 proj_w, proj_b):
    f32 = np.float32
    bf = BF16NP
    q = np.asarray(q, f32); k = np.asarray(k, f32); v = np.asarray(v, f32)
    qpos = np.asarray(qpos, f32); kpos = np.asarray(kpos, f32)
    wq_w = np.asarray(wq_w, f32); wk_w = np.asarray(wk_w, f32)
    wv_w = np.asarray(wv_w, f32); proj_w = np.asarray(proj_w, f32)
    wq_b = np.asarray(wq_b, f32); wk_b = np.asarray(wk_b, f32)
    wv_b = np.asarray(wv_b, f32)

    # [key, query] multiplicative 0/1 pattern of the diagonal 128x128 block
    m2 = np.asarray(mask).reshape(NT, NT)
    patt = np.ascontiguousarray((~m2[0:128, 0:128]).T.astype(bf))

    actT = {}
    for b in range(2):
        actT[("xq", b)] = np.ascontiguousarray((q[b] + qpos[b]).T.astype(bf))
        actT[("xk", b)] = np.ascontiguousarray((k[b] + kpos[b]).T.astype(bf))
        actT[("v", b)] = np.ascontiguousarray(v[b].T.astype(bf))

    in_maps = []
    for c in range(8):
        b, g = divmod(c, 4)
        sl = slice(DS * g, DS * (g + 1))
        in_maps.append({
            "xqT": actT[("xq", b)], "xkT": actT[("xk", b)],
            "vT": actT[("v", b)],
            "wqT": np.ascontiguousarray(
                wq_w[sl, :].T.reshape(NKT, 128, DS).transpose(1, 0, 2).astype(bf)),
            "wkT": np.ascontiguousarray(
                wk_w[sl, :].T.reshape(NKT, 128, DS).transpose(1, 0, 2).astype(bf)),
            "wvT": np.ascontiguousarray(
                wv_w[sl, :].T.reshape(NKT, 128, DS).transpose(1, 0, 2).astype(bf)),
            "wqb": np.ascontiguousarray(wq_b[sl].reshape(2, 128).T),
            "wkb": np.ascontiguousarray(wk_b[sl].reshape(2, 128).T),
            "wvb": np.ascontiguousarray(wv_b[sl].reshape(1, DS).astype(bf)),
            "projT": np.ascontiguousarray(proj_w[:, sl].T.astype(bf)),
            "maskp": patt,
        })
    return in_maps


def kernel(q, k, v, qpos, kpos, mask, wq_w, wq_b, wk_w, wk_b, wv_w, wv_b,
           proj_w, proj_b, _trace=False):
    nc = _get_nc()
    in_maps = make_in_maps(q, k, v, qpos, kpos, mask, wq_w, wq_b, wk_w, wk_b,
                           wv_w, wv_b, proj_w, proj_b)
    res = run_bass_kernel_spmd(nc, in_maps, list(range(8)), trace=_trace)
    if _trace:
        kernel._last_results = res
    out = np.zeros((2, NT, HID), np.float32)
    for c in range(8):
        out[c // 4] += res.results[c]["y"].astype(np.float32)
    out += np.asarray(proj_b, np.float32)[None, None, :]
    return out
